# revision 13
# baseline (speedup 1.0000x reference)
"""Trainium2 Bass kernel for a single-layer causal-attention decoder.

Key observation: VOCAB=5, so Q[i] = QV[x_i] and K[j] = KV[x_j] where
QV/KV are the 5 per-vocab projected rows. The whole [S, S] score matrix
is a gather from the 5x5 Gram table G = QV @ KV.T / sqrt(D):

    scores[i, j] = G[x_i, x_j]

With eg = exp(G) (no max-subtraction needed: |G| < ~6), the causal
softmax-attention collapses to per-vocab prefix counts:

    out[i] = (sum_v eg[x_i, v] * cnt_v(i) * VV[v]) /
             (sum_v eg[x_i, v] * cnt_v(i))

where cnt_v(i) = |{j <= i : x_j = v}|.

v2 layout: everything runs at full 128-partition utilization in
"transposed space". Sequence position i = p*NB + b (p = partition,
b = block 0..NB-1, NB = S/128). All [.., v]-indexed per-position
tensors live as [R, 128] tiles with R = V*NB rows keyed (b, v):

  ohT[(b,v), p]  = (x[p*NB+b] == v)          one DVE is_equal op
  pre[(b,v), p]  = #{b'<=b : x[p*NB+b']=v}   matmul: TBV @ ohT
  tot[(b,v), p]  = #{b'     : x[p*NB+b']=v}  matmul: OB  @ ohT
  off[(b,v), p]  = #{j < p*NB : x_j=v}       one DVE shifted prefix scan
  cnt = off + pre                             one DVE add
  ET[(b,v), p]   = eg[x_i, v]                matmul: EGBD @ ohT
  WT = ET * cnt                               one DVE mult
  out[p,(b,d)]+den = WT.T @ VVBD              3 matmuls (block-diag VV)
  out = num * recip(den); contiguous DMA out (p-major rows)

EGBD = kron(I_NB, eg), TBV = kron(triu_ones, I_5), OB = kron(ones, I_5),
VVBD = block-diag VV with per-chunk grouped denominator columns; eg/VV
are pure functions of the model weights, so they are precomputed
host-side (like the baseline's augmented-weight buffer); the device does
all data-dependent work.

v2.1 scheduling: DMA-completion semaphores post ~1.5-2.3us after the
transfer, so DMA count/placement dominates. All inputs ride in ONE bf16
tensor (x cast to bf16: values 0..4 exact); the compute-critical slice
(x|iota|masks) is issued from the otherwise-idle Scalar queue (its
preamble drains ~1us before Sync's), VVBD from Sync. The three output
chunks are issued on sync/scalar/vector queues so the issues overlap and
the last transfer - whose completion semaphore gates the epilogue -
lands as early as possible.

Sharding: data-parallel over batch. B=8 -> 8 NeuronCores, one sequence
per core; weights replicated. No collectives.
"""

import numpy as np
import ml_dtypes

import concourse.bass as bass
import concourse.mybir as mybir
import concourse.tile as tile
from concourse import bacc
from concourse.bass_utils import run_bass_kernel_spmd

F32 = mybir.dt.float32
BF16 = mybir.dt.bfloat16
F16 = mybir.dt.float16

B = 8
S = 2048
D = 64
V = 5
P = 128
N_CORES = 8


def _dims(S):
    NB = S // P          # sequence blocks per partition
    R = V * NB           # transposed-space rows (b, v)
    # cn columns: xr | iota | EGBD | TBV | OB | VVBD
    C0 = 2 * P + 3 * R
    CW = C0 + (D + 1) * NB
    return NB, R, C0, CW


def _chunks(NB):
    CB = min(NB, 512 // D)
    return [(c0, min(c0 + CB, NB)) for c0 in range(0, NB, CB)]


def _body(tc, aps, S):
    nc = tc.nc
    NB, R, C0, CW = _dims(S)
    cn, out = aps["cn"], aps["out"]
    E = D + 1
    chunks = _chunks(NB)

    from contextlib import ExitStack
    with ExitStack() as ctx:
        consts = ctx.enter_context(tc.tile_pool(name="consts", bufs=1))
        ps3 = ctx.enter_context(tc.tile_pool(name="ps3", bufs=3, space="PSUM"))
        ps_o = ctx.enter_context(
            tc.tile_pool(name="ps_o", bufs=len(chunks), space="PSUM")
        )
        ps_d = ctx.enter_context(tc.tile_pool(name="ps_d", bufs=1, space="PSUM"))

        # ---- tiles ----
        cn_sb = consts.tile([R, CW], BF16)
        ohT = consts.tile([R, P], BF16)
        off = consts.tile([R, P], F32)
        cnt = consts.tile([R, P], F16)
        wt = consts.tile([R, P], BF16)
        rc = consts.tile([P, NB], F32)
        rs_sb = consts.tile([P, NB, D], F32)

        # ---- input DMAs: critical slice on the idle Scalar queue ----
        nc.scalar.dma_start(cn_sb[:, 0:C0], cn[:, 0:C0])
        nc.sync.dma_start(cn_sb[:, C0:CW], cn[:, C0:CW])

        xr = cn_sb[:, 0:P]
        iota = cn_sb[:, P : 2 * P]
        egbd = cn_sb[:, 2 * P : 2 * P + R]
        tbv = cn_sb[:, 2 * P + R : 2 * P + 2 * R]
        ob = cn_sb[:, 2 * P + 2 * R : C0]
        vvbd = cn_sb[:, C0:CW]

        # ---- one-hot straight into transposed space ----
        nc.vector.tensor_tensor(ohT[:], xr, iota, mybir.AluOpType.is_equal)

        # ---- three mask matmuls share rhs = ohT ----
        p_tot = ps3.tile([R, P], F32, tag="t3")
        nc.tensor.matmul(p_tot[:], lhsT=ob, rhs=ohT[:], start=True, stop=True)
        p_pre = ps3.tile([R, P], F32, tag="t3")
        nc.tensor.matmul(p_pre[:], lhsT=tbv, rhs=ohT[:], start=True, stop=True)
        p_et = ps3.tile([R, P], F32, tag="t3")
        nc.tensor.matmul(p_et[:], lhsT=egbd, rhs=ohT[:], start=True, stop=True)

        # ---- cross-partition prefix: full inclusive scan; the pre-mask
        # is TBV-OB host-side so cnt = incl_scan(tot) + (pre - tot) ----
        nc.vector.tensor_tensor_scan(
            off[:], p_tot[:], ohT[:],
            initial=0.0, op0=mybir.AluOpType.add, op1=mybir.AluOpType.bypass,
        )
        nc.vector.tensor_tensor(cnt[:], off[:], p_pre[:], mybir.AluOpType.add)
        nc.vector.tensor_tensor(wt[:], cnt[:], p_et[:], mybir.AluOpType.mult)

        # ---- PV + normalize + store ----
        # denominators for ALL blocks land in their own small PSUM bank
        # (one matmul + ONE reciprocal); numerators pack 8 blocks * 64 =
        # exactly 512 fp32 per bank -> one matmul + one scale-multiply +
        # one contiguous 2KB/partition DMA per chunk
        out_r = out.rearrange("(p b) d -> p b d", b=NB)
        pden = ps_d.tile([P, NB], F32, tag="pden")
        nc.tensor.matmul(
            pden[:], lhsT=wt[:], rhs=vvbd[:, NB * D : NB * E],
            start=True, stop=True,
        )
        nc.vector.reciprocal(rc[:], pden[:])
        dma_engines = [nc.sync, nc.scalar]
        for ci, (c0, c1b) in enumerate(chunks):
            w = c1b - c0
            po = ps_o.tile([P, w * D], F32, tag="po")
            nc.tensor.matmul(
                po[:], lhsT=wt[:], rhs=vvbd[:, c0 * D : c1b * D],
                start=True, stop=True,
            )
            nc.vector.tensor_tensor(
                rs_sb[:, c0:c1b, :], po[:].rearrange("p (b d) -> p b d", d=D),
                rc[:, c0:c1b].unsqueeze(2).to_broadcast((P, w, D)),
                mybir.AluOpType.mult,
            )
            dma_engines[ci % 2].dma_start(out_r[:, c0:c1b, :], rs_sb[:, c0:c1b, :])


def build_nc(S=S, mode=None):
    NB, R, C0, CW = _dims(S)
    nc = bacc.Bacc(trn_type="TRN2", target_bir_lowering=False, debug=False)
    aps = {}
    aps["cn"] = nc.dram_tensor("cn", [R, CW], BF16, kind="ExternalInput").ap()
    aps["out"] = nc.dram_tensor("out", [S, D], F32, kind="ExternalOutput").ap()
    with tile.TileContext(nc) as tc:
        _body(tc, aps, S=S)
    nc.compile()
    return nc


def make_in_maps(x, emb_table, wq, bq, wk, bk, wv, bv, S=S, n_cores=N_CORES):
    NB, R, C0, CW = _dims(S)
    E = D + 1
    x = np.asarray(x)
    emb = np.asarray(emb_table, np.float32)

    # weight-derived tables (parameter preprocessing, host-side)
    QV = emb @ np.asarray(wq, np.float32).T + np.asarray(bq, np.float32)
    KV = emb @ np.asarray(wk, np.float32).T + np.asarray(bk, np.float32)
    VV = emb @ np.asarray(wv, np.float32).T + np.asarray(bv, np.float32)
    eg = np.exp((QV @ KV.T) / np.sqrt(np.float32(D))).astype(np.float32)

    cn = np.zeros((R, CW), np.float32)
    # iota: row r=(b,v) -> v, constant along p
    cn[:, P : 2 * P] = np.tile(np.arange(V, dtype=np.float32), NB)[:, None]
    cn[:, 2 * P : 2 * P + R] = np.kron(np.eye(NB, dtype=np.float32), eg)
    cn[:, 2 * P + R : 2 * P + 2 * R] = np.kron(
        np.triu(np.ones((NB, NB), np.float32))           # A[b',b] = (b'<=b) - 1
        - np.ones((NB, NB), np.float32),
        np.eye(V, dtype=np.float32),
    )
    cn[:, 2 * P + 2 * R : C0] = np.kron(
        np.ones((NB, NB), np.float32), np.eye(V, dtype=np.float32)
    )
    # VVBD: numerator block-diagonal [R, NB*64], then denominator [R, NB]
    for b in range(NB):
        cn[b * V : (b + 1) * V, C0 + b * D : C0 + (b + 1) * D] = VV
        cn[b * V : (b + 1) * V, C0 + NB * D + b] = 1.0

    # per-core x, transposed+replicated: xr[(b,v), p] = x[p*NB + b]
    bf16 = ml_dtypes.bfloat16

    def per_core(xc):
        m = cn.copy()
        xT = np.asarray(xc).reshape(P, NB).T.astype(np.float32)  # [NB, 128]
        m[:, 0:P] = np.repeat(xT, V, axis=0)                     # [(b,v), 128]
        return np.ascontiguousarray(m.astype(bf16))

    return [dict(cn=per_core(np.asarray(x)[c, :S])) for c in range(n_cores)]


_NC_CACHE = {}

MODE = "bf16"  # mask/eg/VV matmuls in bf16; counts exact (f16/f32)


def _get_nc(S=S, mode=None):
    key = S
    if key not in _NC_CACHE:
        _NC_CACHE[key] = build_nc(S=S)
    return _NC_CACHE[key]


def run(inputs, trace=False, **kw):
    in_maps = make_in_maps(**inputs)
    nc = _get_nc()
    res = run_bass_kernel_spmd(nc, in_maps, core_ids=list(range(N_CORES)), trace=trace, **kw)
    out = np.stack([res.results[c]["out"] for c in range(N_CORES)])
    return out, res


def kernel(x, emb_table, wq, bq, wk, bk, wv, bv):
    out, _ = run(dict(x=x, emb_table=emb_table, wq=wq, bq=bq, wk=wk, bk=bk,
                      wv=wv, bv=bv))
    return out


# revision 14
# speedup vs baseline: 1.1546x; 1.1546x over previous
"""Trainium2 Bass kernel for a single-layer causal-attention decoder.

Key observation: VOCAB=5, so Q[i] = QV[x_i] and K[j] = KV[x_j] where
QV/KV are the 5 per-vocab projected rows. The whole [S, S] score matrix
is a gather from the 5x5 Gram table G = QV @ KV.T / sqrt(D):

    scores[i, j] = G[x_i, x_j]

With eg = exp(G) (no max-subtraction needed: |G| < ~6), the causal
softmax-attention collapses to per-vocab prefix counts:

    out[i] = (sum_v eg[x_i, v] * cnt_v(i) * VV[v]) /
             (sum_v eg[x_i, v] * cnt_v(i))

where cnt_v(i) = |{j <= i : x_j = v}|.

v2 layout: everything runs at full 128-partition utilization in
"transposed space". Sequence position i = p*NB + b (p = partition,
b = block 0..NB-1, NB = S/128). All [.., v]-indexed per-position
tensors live as [R, 128] tiles with R = V*NB rows keyed (b, v):

  ohT[(b,v), p]  = (x[p*NB+b] == v)          one DVE is_equal op
  pre[(b,v), p]  = #{b'<=b : x[p*NB+b']=v}   matmul: TBV @ ohT
  tot[(b,v), p]  = #{b'     : x[p*NB+b']=v}  matmul: OB  @ ohT
  off[(b,v), p]  = #{j < p*NB : x_j=v}       one DVE shifted prefix scan
  cnt = off + pre                             one DVE add
  ET[(b,v), p]   = eg[x_i, v]                matmul: EGBD @ ohT
  WT = ET * cnt                               one DVE mult
  den[p, b] (all blocks)  = WT.T @ ones-cols  one tiny matmul, own bank
  num[p,(b,d)] (8 blk/bank) = WT.T @ VVBD     2 matmuls, 512 fp32 = bank
  out = num * recip(den)                      1 recip + 2 DVE mults
  2 contiguous 2KB/partition DMAs out (p-major rows)

EGBD = kron(I_NB, eg), OB = kron(ones, I_5), TBV' = kron(triu-1, I_5)
(0/-1 entries: cnt = incl_scan(tot) + (pre - tot), no memset needed),
VVBD = block-diag VV plus separate denominator ones-columns; eg/VV are
pure functions of the model weights, so they are precomputed host-side
(like the baseline's augmented-weight buffer); the device does all
data-dependent work.

Scheduling: DMA-completion semaphores post ~1.5-2.4us after the
transfer, and each dma_start costs ~700ns of queue-issue time, so DMA
count/placement dominates. All inputs ride in ONE bf16 tensor (x cast
to bf16: values 0..4 exact); the compute-critical slice (x|iota|masks)
is issued from the otherwise-idle Scalar queue, VVBD from Sync. The two
output chunks are issued on sync/scalar so the issues overlap and the
last transfer - whose completion semaphore gates the epilogue - lands
as early as possible. The denominator matmul goes first so the single
reciprocal hides under the first numerator matmul. Measured: 26.0us
(baseline) -> 18.6us at matched device clock; the residue is ~7us boot
preamble, ~2.4us input-sem latency, ~4.6us compute window, ~4.3us
output-sem + epilogue.

Sharding: data-parallel over batch. B=8 -> 8 NeuronCores, one sequence
per core; weights replicated. No collectives.
"""

import numpy as np
import ml_dtypes

import concourse.bass as bass
import concourse.mybir as mybir
import concourse.tile as tile
from concourse import bacc
from concourse.bass_utils import run_bass_kernel_spmd

F32 = mybir.dt.float32
BF16 = mybir.dt.bfloat16
F16 = mybir.dt.float16

B = 8
S = 2048
D = 64
V = 5
P = 128
N_CORES = 8


def _dims(S):
    NB = S // P          # sequence blocks per partition
    R = V * NB           # transposed-space rows (b, v)
    # cn columns: xr | iota | EGBD | TBV | OB | VVBD
    C0 = 2 * P + 3 * R
    CW = C0 + (D + 1) * NB
    return NB, R, C0, CW


def _chunks(NB):
    CB = min(NB, 512 // D)
    return [(c0, min(c0 + CB, NB)) for c0 in range(0, NB, CB)]


def _body(tc, aps, S):
    nc = tc.nc
    NB, R, C0, CW = _dims(S)
    cn, out = aps["cn"], aps["out"]
    E = D + 1
    chunks = _chunks(NB)

    from contextlib import ExitStack
    with ExitStack() as ctx:
        consts = ctx.enter_context(tc.tile_pool(name="consts", bufs=1))
        ps3 = ctx.enter_context(tc.tile_pool(name="ps3", bufs=3, space="PSUM"))
        ps_o = ctx.enter_context(
            tc.tile_pool(name="ps_o", bufs=len(chunks), space="PSUM")
        )
        ps_d = ctx.enter_context(tc.tile_pool(name="ps_d", bufs=1, space="PSUM"))

        # ---- tiles ----
        cn_sb = consts.tile([R, CW], BF16)
        ohT = consts.tile([R, P], BF16)
        off = consts.tile([R, P], F32)
        cnt = consts.tile([R, P], F16)
        wt = consts.tile([R, P], BF16)
        rc = consts.tile([P, NB], F32)
        rs_sb = consts.tile([P, NB, D], F32)

        # ---- input DMAs: critical slice on the idle Scalar queue ----
        nc.scalar.dma_start(cn_sb[:, 0:C0], cn[:, 0:C0])
        nc.sync.dma_start(cn_sb[:, C0:CW], cn[:, C0:CW])

        xr = cn_sb[:, 0:P]
        iota = cn_sb[:, P : 2 * P]
        egbd = cn_sb[:, 2 * P : 2 * P + R]
        tbv = cn_sb[:, 2 * P + R : 2 * P + 2 * R]
        ob = cn_sb[:, 2 * P + 2 * R : C0]
        vvbd = cn_sb[:, C0:CW]

        # ---- one-hot straight into transposed space ----
        nc.vector.tensor_tensor(ohT[:], xr, iota, mybir.AluOpType.is_equal)

        # ---- three mask matmuls share rhs = ohT ----
        p_tot = ps3.tile([R, P], F32, tag="t3")
        nc.tensor.matmul(p_tot[:], lhsT=ob, rhs=ohT[:], start=True, stop=True)
        p_pre = ps3.tile([R, P], F32, tag="t3")
        nc.tensor.matmul(p_pre[:], lhsT=tbv, rhs=ohT[:], start=True, stop=True)
        p_et = ps3.tile([R, P], F32, tag="t3")
        nc.tensor.matmul(p_et[:], lhsT=egbd, rhs=ohT[:], start=True, stop=True)

        # ---- cross-partition prefix: full inclusive scan; the pre-mask
        # is TBV-OB host-side so cnt = incl_scan(tot) + (pre - tot) ----
        nc.vector.tensor_tensor_scan(
            off[:], p_tot[:], ohT[:],
            initial=0.0, op0=mybir.AluOpType.add, op1=mybir.AluOpType.bypass,
        )
        nc.vector.tensor_tensor(cnt[:], off[:], p_pre[:], mybir.AluOpType.add)
        nc.vector.tensor_tensor(wt[:], cnt[:], p_et[:], mybir.AluOpType.mult)

        # ---- PV + normalize + store ----
        # denominators for ALL blocks land in their own small PSUM bank
        # (one matmul + ONE reciprocal); numerators pack 8 blocks * 64 =
        # exactly 512 fp32 per bank -> one matmul + one scale-multiply +
        # one contiguous 2KB/partition DMA per chunk
        out_r = out.rearrange("(p b) d -> p b d", b=NB)
        pden = ps_d.tile([P, NB], F32, tag="pden")
        nc.tensor.matmul(
            pden[:], lhsT=wt[:], rhs=vvbd[:, NB * D : NB * E],
            start=True, stop=True,
        )
        nc.vector.reciprocal(rc[:], pden[:])
        dma_engines = [nc.sync, nc.scalar]
        for ci, (c0, c1b) in enumerate(chunks):
            w = c1b - c0
            po = ps_o.tile([P, w * D], F32, tag="po")
            nc.tensor.matmul(
                po[:], lhsT=wt[:], rhs=vvbd[:, c0 * D : c1b * D],
                start=True, stop=True,
            )
            nc.vector.tensor_tensor(
                rs_sb[:, c0:c1b, :], po[:].rearrange("p (b d) -> p b d", d=D),
                rc[:, c0:c1b].unsqueeze(2).to_broadcast((P, w, D)),
                mybir.AluOpType.mult,
            )
            dma_engines[ci % 2].dma_start(out_r[:, c0:c1b, :], rs_sb[:, c0:c1b, :])


def build_nc(S=S, mode=None):
    NB, R, C0, CW = _dims(S)
    nc = bacc.Bacc(trn_type="TRN2", target_bir_lowering=False, debug=False)
    aps = {}
    aps["cn"] = nc.dram_tensor("cn", [R, CW], BF16, kind="ExternalInput").ap()
    aps["out"] = nc.dram_tensor("out", [S, D], F32, kind="ExternalOutput").ap()
    with tile.TileContext(nc) as tc:
        _body(tc, aps, S=S)
    nc.compile()
    return nc


def make_in_maps(x, emb_table, wq, bq, wk, bk, wv, bv, S=S, n_cores=N_CORES):
    NB, R, C0, CW = _dims(S)
    E = D + 1
    x = np.asarray(x)
    emb = np.asarray(emb_table, np.float32)

    # weight-derived tables (parameter preprocessing, host-side)
    QV = emb @ np.asarray(wq, np.float32).T + np.asarray(bq, np.float32)
    KV = emb @ np.asarray(wk, np.float32).T + np.asarray(bk, np.float32)
    VV = emb @ np.asarray(wv, np.float32).T + np.asarray(bv, np.float32)
    eg = np.exp((QV @ KV.T) / np.sqrt(np.float32(D))).astype(np.float32)

    cn = np.zeros((R, CW), np.float32)
    # iota: row r=(b,v) -> v, constant along p
    cn[:, P : 2 * P] = np.tile(np.arange(V, dtype=np.float32), NB)[:, None]
    cn[:, 2 * P : 2 * P + R] = np.kron(np.eye(NB, dtype=np.float32), eg)
    cn[:, 2 * P + R : 2 * P + 2 * R] = np.kron(
        np.triu(np.ones((NB, NB), np.float32))           # A[b',b] = (b'<=b) - 1
        - np.ones((NB, NB), np.float32),
        np.eye(V, dtype=np.float32),
    )
    cn[:, 2 * P + 2 * R : C0] = np.kron(
        np.ones((NB, NB), np.float32), np.eye(V, dtype=np.float32)
    )
    # VVBD: numerator block-diagonal [R, NB*64], then denominator [R, NB]
    for b in range(NB):
        cn[b * V : (b + 1) * V, C0 + b * D : C0 + (b + 1) * D] = VV
        cn[b * V : (b + 1) * V, C0 + NB * D + b] = 1.0

    # per-core x, transposed+replicated: xr[(b,v), p] = x[p*NB + b]
    bf16 = ml_dtypes.bfloat16

    def per_core(xc):
        m = cn.copy()
        xT = np.asarray(xc).reshape(P, NB).T.astype(np.float32)  # [NB, 128]
        m[:, 0:P] = np.repeat(xT, V, axis=0)                     # [(b,v), 128]
        return np.ascontiguousarray(m.astype(bf16))

    return [dict(cn=per_core(np.asarray(x)[c, :S])) for c in range(n_cores)]


_NC_CACHE = {}

MODE = "bf16"  # mask/eg/VV matmuls in bf16; counts exact (f16/f32)


def _get_nc(S=S, mode=None):
    key = S
    if key not in _NC_CACHE:
        _NC_CACHE[key] = build_nc(S=S)
    return _NC_CACHE[key]


def run(inputs, trace=False, **kw):
    in_maps = make_in_maps(**inputs)
    nc = _get_nc()
    res = run_bass_kernel_spmd(nc, in_maps, core_ids=list(range(N_CORES)), trace=trace, **kw)
    out = np.stack([res.results[c]["out"] for c in range(N_CORES)])
    return out, res


def kernel(x, emb_table, wq, bq, wk, bk, wv, bv):
    out, _ = run(dict(x=x, emb_table=emb_table, wq=wq, bq=bq, wk=wk, bk=bk,
                      wv=wv, bv=bv))
    return out
